# revision 3
# baseline (speedup 1.0000x reference)
"""Depthwise 4x4 FIR blur (upfirdn2d pad=(2,1)) on 8 Trainium2 NeuronCores.

Full-bf16 pipeline (fits the rel-err < 2e-2 gate with ~4x margin).

Strategy
--------
Data parallel: shard batch N=32 -> 4 per core; 1024 independent 64x64
images per core. All device-side data is bf16 (inputs quantized on the
host, outputs cast back to fp32 on the host): halves HBM traffic vs
fp32 AND runs the TensorEngine at 1 cycle/row instead of fp32's 4.

Per-core compute: separable SVD decomposition of the 4x4 tap kernel
into rank-1 terms; each term is two banded-Toeplitz matmuls:

  pass 1 (H-conv):  ps1 = X.T @ TC2      (X stationary, TC2 moving)
  pass 2 (W-conv):  ps2 += TR2.T @ Q     (TR2 stationary, Q moving)

X packs 16 images per supergroup [128 x 512] (partition = kb*64+row,
free = q*64+col).  Pass 1 runs 4 matmuls of N=128 per supergroup with
the data as the stationary operand; its PSUM result holds H-convolved
images transposed (W on partitions).  That puts the W index on the
partition (contraction) dim, so pass 2 uses the constant Toeplitz TR2
as the stationary operand and streams Q in a single N=512 matmul: no
per-matmul weight reloads of data, W-conv lands in PSUM with each
image fully transposed.  The host-side unpermute absorbs the
transposed layout at zero device cost.

PSUM->SBUF drains (the only elementwise work) are whole-instruction
role-split: DVE copies ps1->Q, ACT copies ps2->O, both casting
fp32->bf16.  One instruction per copy per supergroup minimizes the
fixed per-instruction engine overheads.

I/O: host pre-permutes x into a [128, 32768] bf16 per-core layout
(every DMA slab contiguous per partition, 1 MiB per dma_start across
all 128 partitions); inverse permutation + fp32 cast on the host.

Pipelining: pass 2 runs a couple supergroups behind pass 1 (software
skew) so the TensorEngine never stalls on the PSUM->SBUF drains.
"""

import functools
import math

import ml_dtypes
import numpy as np

import concourse.bacc as bacc
import concourse.tile as tile
from concourse import mybir
from concourse.bass_utils import run_bass_kernel_spmd

BF16 = np.dtype(ml_dtypes.bfloat16)

N_CORES = 8
N, C, H, W = 32, 256, 64, 64
PER_CORE = N // N_CORES        # 4 batch entries per core
IMGS = PER_CORE * C            # 1024 images per core
SG = 16                        # images per supergroup
NSG = IMGS // SG               # 64 supergroups per core
SPB = 4                        # supergroups per DMA slab (512 KiB bf16)
NSLAB = NSG // SPB


def _toeplitz64(vec4):
    """T[p, i] = vec4[1 + i - p] when 0 <= 1+i-p <= 3 else 0 ([64, 64])."""
    T = np.zeros((64, 64), np.float64)
    for a in range(4):
        k = a - 1
        T += np.diag(np.full(64 - abs(k), vec4[a]), k=k)
    return T


def _permute_in(x_core):
    """[1024, 64, 64] -> [128, NSG*512] bf16 host layout.

    Supergroup s holds images 16s..16s+15; image (kb, q) = 16s + 8kb + q
    lands at SBUF partition 64kb+row, free s*512 + q*64 + col."""
    v = x_core.reshape(NSG, 2, 8, 64, 64)           # [s, kb, q, p, w]
    v = v.transpose(1, 3, 0, 2, 4)                  # [kb, p, s, q, w]
    return np.ascontiguousarray(v.reshape(128, NSG * 512)).astype(BF16)


def _permute_out(o_perm):
    """Inverse layout map for the (transposed) output buffer -> fp32.

    Image (kb, q=2t+qq) of supergroup s sits at
    op[qq*64 + wout, s*512 + t*128 + kb*64 + hout]."""
    v = o_perm.reshape(2, 64, NSG, 4, 2, 64)        # [qq, wout, s, t, kb, hout]
    v = v.transpose(2, 4, 3, 0, 5, 1)               # [s, kb, t, qq, hout, wout]
    return np.ascontiguousarray(v).astype(np.float32).reshape(IMGS, 64, 64)


@functools.lru_cache(maxsize=32)
def _build(rank, loops=1, dyn_loop=False, xbufs=4, obufs=4, qbufs=0,
           ps1b=2, ps2b=2, skew=2, spb=SPB):
    """Build + compile the per-core bass program (same NEFF on all cores).

    dyn_loop=True (benchmark-only) wraps the computation in a hardware
    For_i loop whose trip count comes from an extra `nrep` input."""
    import concourse.bass as bass
    nc = bacc.Bacc("TRN2", target_bir_lowering=False, debug=False)
    dt = mybir.dt.bfloat16
    ft = mybir.dt.float32
    xp = nc.dram_tensor("xp", [128, NSG * 512], dt, kind="ExternalInput").ap()
    tcol = nc.dram_tensor("tcol", [rank, 128, 128], dt, kind="ExternalInput").ap()
    trow = nc.dram_tensor("trow", [rank, 128, 128], dt, kind="ExternalInput").ap()
    op = nc.dram_tensor("op", [128, NSG * 512], dt, kind="ExternalOutput").ap()
    if dyn_loop:
        nrep = nc.dram_tensor("nrep", [1, 1], mybir.dt.int32,
                              kind="ExternalInput").ap()

    with tile.TileContext(nc) as tc:
        with (
            tc.tile_pool(name="consts", bufs=1) as cpool,
            tc.tile_pool(name="xin", bufs=xbufs) as xpool,
            tc.tile_pool(name="q", bufs=qbufs or max(8, (skew + 2) * rank)) as qpool,
            tc.tile_pool(name="o", bufs=obufs) as opool,
            tc.tile_pool(name="ps1", bufs=ps1b, space="PSUM") as ps1pool,
            tc.tile_pool(name="ps2", bufs=ps2b, space="PSUM") as ps2pool,
        ):
            # variable slab plan (units: supergroups): small slabs at both
            # ends shorten the pipeline fill (compute starts after a small
            # first DMA) and the drain (the last output chase is short);
            # bulk slabs amortize per-DMA overheads
            assert (NSG - 8) % spb == 0, (NSG, spb)
            plan = [2, 2] + [spb] * ((NSG - 8) // spb) + [2, 2]

            # first input slab goes ahead of the (tiny) const loads on the
            # SP ring: pass 1 cannot start before the slab lands anyway, so
            # the consts ride in its shadow (For_i bench builds load consts
            # outside the loop, where their position is irrelevant)
            X0 = None
            if not dyn_loop:
                X0 = xpool.tile([128, spb * 512], dt, tag="X")
                nc.sync.dma_start(X0[:, :plan[0] * 512],
                                  xp[:, 0:plan[0] * 512])
            tcs, trs = [], []
            for r in range(rank):
                tct = cpool.tile([128, 128], dt, tag=f"tc{r}")
                nc.sync.dma_start(tct[:], tcol[r])
                trt = cpool.tile([128, 128], dt, tag=f"tr{r}")
                nc.sync.dma_start(trt[:], trow[r])
                tcs.append(tct)
                trs.append(trt)

            import contextlib
            loop_cm = contextlib.nullcontext()
            if dyn_loop:
                cnt = cpool.tile([1, 1], mybir.dt.int32, tag="cnt")
                cnt_sem = nc.alloc_semaphore("cnt_sem")
                with tc.tile_critical():
                    nc.sync.dma_start(cnt[:], nrep[:]).then_inc(cnt_sem, 16)
                    regs = []
                    for e in mybir.ALL_ENGINES:
                        rr = nc.alloc_register(e, f"cnt_{e.name}")
                        nc.engines[e].reg_load(rr, cnt[0:1, 0:1])._wait_ge(
                            cnt_sem, 16)
                        regs.append(rr)
                rv = nc.snap(bass.RegisterHandles(regs))
                loop_cm = tc.For_i(0, rv, 1)

            # supergroups are processed in PAIRS (1024 free columns): the
            # PSUM->SBUF drains then move 1024 elements per instruction,
            # amortizing the fixed DVE/ACT access overheads.  PSUM tiles
            # span 2 banks; every matmul still writes within one bank.
            slabs = {}     # slab idx -> [X, O, n_pairs_done, sg_off, sz_sg]
            pending = []   # (slab idx, pr, [Q_r ...]) awaiting pass 2

            def do_pass1(k, pr):
                X = slabs[k][0]
                qs = []
                for r in range(rank):
                    ps1 = ps1pool.tile([128, 1024], ft, tag="ps1")
                    for t in range(8):
                        nc.tensor.matmul(
                            ps1[:, 128 * t:128 * (t + 1)],
                            X[:, pr * 1024 + 128 * t: pr * 1024 + 128 * (t + 1)],
                            tcs[r][:], start=True, stop=True)
                    Q = qpool.tile([128, 1024], dt, tag="Q")
                    nc.vector.tensor_copy(Q[:], ps1[:])
                    qs.append(Q)
                pending.append((k, pr, qs))

            def do_pass2():
                k, pr, qs = pending.pop(0)
                ps2 = ps2pool.tile([128, 1024], ft, tag="ps2")
                for t in range(2):
                    for r in range(rank):
                        nc.tensor.matmul(
                            ps2[:, 512 * t:512 * (t + 1)], trs[r][:],
                            qs[r][:, 512 * t:512 * (t + 1)],
                            start=(r == 0), stop=(r == rank - 1))
                O = slabs[k][1]
                nc.scalar.copy(O[:, pr * 1024:(pr + 1) * 1024], ps2[:])
                slabs[k][2] += 1
                if slabs[k][2] * 2 == slabs[k][4]:
                    sg_off, sz = slabs[k][3], slabs[k][4]
                    nc.scalar.dma_start(
                        op[:, sg_off * 512:(sg_off + sz) * 512],
                        O[:, :sz * 512])
                    del slabs[k]

            with loop_cm:
                for rep in range(loops):
                    sg_off = 0
                    for j, sz in enumerate(plan):
                        k = rep * len(plan) + j
                        if k == 0 and X0 is not None:
                            X = X0
                        else:
                            X = xpool.tile([128, spb * 512], dt, tag="X")
                            nc.sync.dma_start(
                                X[:, :sz * 512],
                                xp[:, sg_off * 512:(sg_off + sz) * 512])
                        O = opool.tile([128, spb * 512], dt, tag="O")
                        slabs[k] = [X, O, 0, sg_off, sz]
                        for pr in range(sz // 2):
                            # drain an eligible pass 2 BEFORE the next
                            # pass 1: a new slab's first pass-1 matmul waits
                            # on its input DMA, and ready pass-2 work must
                            # not sit behind that wait in the PE's in-order
                            # instruction stream
                            if len(pending) > skew:
                                do_pass2()
                            do_pass1(k, pr)
                        sg_off += sz
                while pending:
                    do_pass2()
    nc.compile()
    return nc


def _decompose(k):
    """SVD rank decomposition of the 4x4 tap kernel into blockdiag
    Toeplitz constant pairs (tcol[r], trow[r]) of shape [128, 128] bf16."""
    U, S, Vt = np.linalg.svd(np.asarray(k, np.float64))
    rank = max(1, int((S > S[0] * 1e-9).sum())) if S[0] > 0 else 1
    tcs = np.zeros((rank, 128, 128), np.float64)
    trs = np.zeros((rank, 128, 128), np.float64)
    for r in range(rank):
        u = U[:, r] * math.sqrt(S[r])
        v = Vt[r, :] * math.sqrt(S[r])
        Tc = _toeplitz64(u)
        Tr = _toeplitz64(v)
        tcs[r, :64, :64] = Tc
        tcs[r, 64:, 64:] = Tc
        trs[r, :64, :64] = Tr
        trs[r, 64:, 64:] = Tr
    return tcs.astype(BF16), trs.astype(BF16)


def run(x, k, trace=False, loops=1):
    """Run the blur on 8 cores. Returns (out, BassKernelResults)."""
    x = np.asarray(x, dtype=np.float32)
    k = np.asarray(k, dtype=np.float32)
    assert x.shape == (N, C, H, W), x.shape
    assert k.shape == (4, 4), k.shape
    tcs, trs = _decompose(k)
    nc = _build(tcs.shape[0], loops)
    in_maps = [
        {
            "xp": _permute_in(x[i * PER_CORE:(i + 1) * PER_CORE].reshape(IMGS, H, W)),
            "tcol": tcs,
            "trow": trs,
        }
        for i in range(N_CORES)
    ]
    res = run_bass_kernel_spmd(nc, in_maps, core_ids=list(range(N_CORES)),
                               trace=trace)
    out = np.concatenate(
        [
            _permute_out(r["op"]).reshape(PER_CORE, C, H, W)
            for r in res.results
        ],
        axis=0,
    )
    return out, res


def kernel(x, kernel):
    return run(x, kernel)[0]


# revision 5
# speedup vs baseline: 1.4942x; 1.4942x over previous
"""Depthwise 4x4 FIR blur (upfirdn2d pad=(2,1)) on 8 Trainium2 NeuronCores.

Full-bf16 pipeline (fits the rel-err < 2e-2 gate with ~4x margin).

Strategy
--------
Data parallel: shard batch N=32 -> 4 per core; 1024 independent 64x64
images per core. All device-side data is bf16 (inputs quantized on the
host, outputs cast back to fp32 on the host): halves HBM traffic vs
fp32 AND runs the TensorEngine at 1 cycle/row instead of fp32's 4.

Per-core compute: separable SVD decomposition of the 4x4 tap kernel
into rank-1 terms; each term is two banded-Toeplitz matmuls:

  pass 1 (H-conv):  ps1 = X.T @ TC2      (X stationary, TC2 moving)
  pass 2 (W-conv):  ps2 += TR2.T @ Q     (TR2 stationary, Q moving)

X packs 16 images per supergroup [128 x 512] (partition = kb*64+row,
free = q*64+col).  Pass 1 runs 4 matmuls of N=128 per supergroup with
the data as the stationary operand; its PSUM result holds H-convolved
images transposed (W on partitions).  That puts the W index on the
partition (contraction) dim, so pass 2 uses the constant Toeplitz TR2
as the stationary operand and streams Q in a single N=512 matmul: no
per-matmul weight reloads of data, W-conv lands in PSUM with each
image fully transposed.  The host-side unpermute absorbs the
transposed layout at zero device cost.

PSUM->SBUF drains (the only elementwise work) are whole-instruction
role-split: DVE copies ps1->Q, ACT copies ps2->O, both casting
fp32->bf16.  One instruction per copy per supergroup minimizes the
fixed per-instruction engine overheads.

I/O: host pre-permutes x into a [128, 32768] bf16 per-core layout
(every DMA slab contiguous per partition, 1 MiB per dma_start across
all 128 partitions); inverse permutation + fp32 cast on the host.

Pipelining: pass 2 runs a couple supergroups behind pass 1 (software
skew) so the TensorEngine never stalls on the PSUM->SBUF drains.
"""

import functools
import math

import ml_dtypes
import numpy as np

import concourse.bacc as bacc
import concourse.tile as tile
from concourse import mybir
from concourse.bass_utils import run_bass_kernel_spmd

BF16 = np.dtype(ml_dtypes.bfloat16)

N_CORES = 8
N, C, H, W = 32, 256, 64, 64
PER_CORE = N // N_CORES        # 4 batch entries per core
IMGS = PER_CORE * C            # 1024 images per core
SG = 16                        # images per supergroup
NSG = IMGS // SG               # 64 supergroups per core
SPB = 4                        # supergroups per DMA slab (512 KiB bf16)
NSLAB = NSG // SPB


def _toeplitz64(vec4):
    """T[p, i] = vec4[1 + i - p] when 0 <= 1+i-p <= 3 else 0 ([64, 64])."""
    T = np.zeros((64, 64), np.float64)
    for a in range(4):
        k = a - 1
        T += np.diag(np.full(64 - abs(k), vec4[a]), k=k)
    return T


def _permute_in(x_core):
    """[1024, 64, 64] -> [128, NSG*512] bf16 host layout.

    Supergroup s holds images 16s..16s+15; image (kb, q) = 16s + 8kb + q
    lands at SBUF partition 64kb+row, free s*512 + q*64 + col."""
    v = x_core.reshape(NSG, 2, 8, 64, 64)           # [s, kb, q, p, w]
    v = v.transpose(1, 3, 0, 2, 4)                  # [kb, p, s, q, w]
    return np.ascontiguousarray(v.reshape(128, NSG * 512)).astype(BF16)


def _permute_out(o_perm):
    """Inverse layout map for the (transposed) output buffer -> fp32.

    Image (kb, q=2t+qq) of supergroup s sits at
    op[qq*64 + wout, s*512 + t*128 + kb*64 + hout]."""
    v = o_perm.reshape(2, 64, NSG, 4, 2, 64)        # [qq, wout, s, t, kb, hout]
    v = v.transpose(2, 4, 3, 0, 5, 1)               # [s, kb, t, qq, hout, wout]
    return np.ascontiguousarray(v).astype(np.float32).reshape(IMGS, 64, 64)


@functools.lru_cache(maxsize=32)
def _build(rank, loops=1, dyn_loop=False, xbufs=4, obufs=4, qbufs=0,
           ps1b=2, ps2b=2, skew=2, spb=SPB):
    """Build + compile the per-core bass program (same NEFF on all cores).

    dyn_loop=True (benchmark-only) wraps the computation in a hardware
    For_i loop whose trip count comes from an extra `nrep` input."""
    import concourse.bass as bass
    nc = bacc.Bacc("TRN2", target_bir_lowering=False, debug=False)
    dt = mybir.dt.bfloat16
    ft = mybir.dt.float32
    xp = nc.dram_tensor("xp", [128, NSG * 512], dt, kind="ExternalInput").ap()
    tcol = nc.dram_tensor("tcol", [rank, 128, 128], dt, kind="ExternalInput").ap()
    trow = nc.dram_tensor("trow", [rank, 128, 128], dt, kind="ExternalInput").ap()
    op = nc.dram_tensor("op", [128, NSG * 512], dt, kind="ExternalOutput").ap()
    if dyn_loop:
        nrep = nc.dram_tensor("nrep", [1, 1], mybir.dt.int32,
                              kind="ExternalInput").ap()

    with tile.TileContext(nc) as tc:
        with (
            tc.tile_pool(name="consts", bufs=1) as cpool,
            tc.tile_pool(name="xin", bufs=xbufs) as xpool,
            tc.tile_pool(name="q", bufs=qbufs or max(8, (skew + 2) * rank)) as qpool,
            tc.tile_pool(name="o", bufs=obufs) as opool,
            tc.tile_pool(name="ps1", bufs=ps1b, space="PSUM") as ps1pool,
            tc.tile_pool(name="ps2", bufs=ps2b, space="PSUM") as ps2pool,
        ):
            # variable slab plan (units: supergroups): small slabs at both
            # ends shorten the pipeline fill (compute starts after a small
            # first DMA) and the drain (the last output chase is short);
            # bulk slabs amortize per-DMA overheads
            assert (NSG - 8) % spb == 0, (NSG, spb)
            plan = [2, 2] + [spb] * ((NSG - 8) // spb) + [2, 2]

            # first input slab goes ahead of the (tiny) const loads on the
            # SP ring: pass 1 cannot start before the slab lands anyway, so
            # the consts ride in its shadow (For_i bench builds load consts
            # outside the loop, where their position is irrelevant)
            X0 = None
            if not dyn_loop:
                X0 = xpool.tile([128, spb * 512], dt, tag="X")
                nc.sync.dma_start(X0[:, :plan[0] * 512],
                                  xp[:, 0:plan[0] * 512])
            tcs, trs = [], []
            for r in range(rank):
                tct = cpool.tile([128, 128], dt, tag=f"tc{r}")
                nc.sync.dma_start(tct[:], tcol[r])
                trt = cpool.tile([128, 128], dt, tag=f"tr{r}")
                nc.sync.dma_start(trt[:], trow[r])
                tcs.append(tct)
                trs.append(trt)

            import contextlib
            loop_cm = contextlib.nullcontext()
            if dyn_loop:
                cnt = cpool.tile([1, 1], mybir.dt.int32, tag="cnt")
                cnt_sem = nc.alloc_semaphore("cnt_sem")
                with tc.tile_critical():
                    nc.sync.dma_start(cnt[:], nrep[:]).then_inc(cnt_sem, 16)
                    regs = []
                    for e in mybir.ALL_ENGINES:
                        rr = nc.alloc_register(e, f"cnt_{e.name}")
                        nc.engines[e].reg_load(rr, cnt[0:1, 0:1])._wait_ge(
                            cnt_sem, 16)
                        regs.append(rr)
                rv = nc.snap(bass.RegisterHandles(regs))
                loop_cm = tc.For_i(0, rv, 1)

            # supergroups are processed in PAIRS (1024 free columns): the
            # PSUM->SBUF drains then move 1024 elements per instruction,
            # amortizing the fixed DVE/ACT access overheads.  PSUM tiles
            # span 2 banks; every matmul still writes within one bank.
            slabs = {}     # slab idx -> [X, O, n_pairs_done, sg_off, sz_sg]
            pending = []   # (slab idx, pr, [Q_r ...]) awaiting pass 2

            def do_pass1(k, pr):
                X = slabs[k][0]
                qs = []
                for r in range(rank):
                    ps1 = ps1pool.tile([128, 1024], ft, tag="ps1")
                    for t in range(8):
                        nc.tensor.matmul(
                            ps1[:, 128 * t:128 * (t + 1)],
                            X[:, pr * 1024 + 128 * t: pr * 1024 + 128 * (t + 1)],
                            tcs[r][:], start=True, stop=True)
                    Q = qpool.tile([128, 1024], dt, tag="Q")
                    nc.vector.tensor_copy(Q[:], ps1[:])
                    qs.append(Q)
                pending.append((k, pr, qs))

            def do_pass2():
                k, pr, qs = pending.pop(0)
                ps2 = ps2pool.tile([128, 1024], ft, tag="ps2")
                for t in range(2):
                    for r in range(rank):
                        nc.tensor.matmul(
                            ps2[:, 512 * t:512 * (t + 1)], trs[r][:],
                            qs[r][:, 512 * t:512 * (t + 1)],
                            start=(r == 0), stop=(r == rank - 1))
                O = slabs[k][1]
                nc.scalar.copy(O[:, pr * 1024:(pr + 1) * 1024], ps2[:])
                slabs[k][2] += 1
                if slabs[k][2] * 2 == slabs[k][4]:
                    sg_off, sz = slabs[k][3], slabs[k][4]
                    nc.scalar.dma_start(
                        op[:, sg_off * 512:(sg_off + sz) * 512],
                        O[:, :sz * 512])
                    del slabs[k]

            with loop_cm:
                for rep in range(loops):
                    sg_off = 0
                    for j, sz in enumerate(plan):
                        k = rep * len(plan) + j
                        if k == 0 and X0 is not None:
                            X = X0
                        else:
                            X = xpool.tile([128, spb * 512], dt, tag="X")
                            nc.sync.dma_start(
                                X[:, :sz * 512],
                                xp[:, sg_off * 512:(sg_off + sz) * 512])
                        O = opool.tile([128, spb * 512], dt, tag="O")
                        slabs[k] = [X, O, 0, sg_off, sz]
                        # taper the software skew on the final slabs so the
                        # end-of-kernel pass-2/drain chain is short
                        cur_skew = skew if j < len(plan) - 2 else 0
                        for pr in range(sz // 2):
                            # drain an eligible pass 2 BEFORE the next
                            # pass 1: a new slab's first pass-1 matmul waits
                            # on its input DMA, and ready pass-2 work must
                            # not sit behind that wait in the PE's in-order
                            # instruction stream
                            while len(pending) > cur_skew:
                                do_pass2()
                            do_pass1(k, pr)
                        sg_off += sz
                while pending:
                    do_pass2()
    nc.compile()
    return nc


def _decompose(k):
    """SVD rank decomposition of the 4x4 tap kernel into blockdiag
    Toeplitz constant pairs (tcol[r], trow[r]) of shape [128, 128] bf16."""
    U, S, Vt = np.linalg.svd(np.asarray(k, np.float64))
    rank = max(1, int((S > S[0] * 1e-9).sum())) if S[0] > 0 else 1
    tcs = np.zeros((rank, 128, 128), np.float64)
    trs = np.zeros((rank, 128, 128), np.float64)
    for r in range(rank):
        u = U[:, r] * math.sqrt(S[r])
        v = Vt[r, :] * math.sqrt(S[r])
        Tc = _toeplitz64(u)
        Tr = _toeplitz64(v)
        tcs[r, :64, :64] = Tc
        tcs[r, 64:, 64:] = Tc
        trs[r, :64, :64] = Tr
        trs[r, 64:, 64:] = Tr
    return tcs.astype(BF16), trs.astype(BF16)


def run(x, k, trace=False, loops=1):
    """Run the blur on 8 cores. Returns (out, BassKernelResults)."""
    x = np.asarray(x, dtype=np.float32)
    k = np.asarray(k, dtype=np.float32)
    assert x.shape == (N, C, H, W), x.shape
    assert k.shape == (4, 4), k.shape
    tcs, trs = _decompose(k)
    nc = _build(tcs.shape[0], loops)
    in_maps = [
        {
            "xp": _permute_in(x[i * PER_CORE:(i + 1) * PER_CORE].reshape(IMGS, H, W)),
            "tcol": tcs,
            "trow": trs,
        }
        for i in range(N_CORES)
    ]
    res = run_bass_kernel_spmd(nc, in_maps, core_ids=list(range(N_CORES)),
                               trace=trace)
    out = np.concatenate(
        [
            _permute_out(r["op"]).reshape(PER_CORE, C, H, W)
            for r in res.results
        ],
        axis=0,
    )
    return out, res


def kernel(x, kernel):
    return run(x, kernel)[0]
